# revision 30
# baseline (speedup 1.0000x reference)
"""Trainium2 Bass kernel for nn_Bottleneck_SAA (CSP bottleneck + dual PAM attention).

Sharding: 8 cores = 4 batches x 2 row-halves. One SPMD program; odd cores
receive a vertically flipped image + vertically flipped conv kernels, so
every core computes output rows 0..31 of its (possibly flipped) input
(conv(flip(x), flip_h(w)) == flip(conv(x, w)); attention is invariant to
permuting the softmax axis). The host flips those outputs back.

Per-core on-chip pipeline (fp16 matmul operands, fp32 PSUM accumulate):
  conv1(3x3, BN+SiLU folded into weights/bias) -> conv2 -> q/k/vT
  projections -> flash-style attention in [m, n] orientation:
    energyT = k_chunk^T q  (PSUM) -> exp (ACT, fp16 out) ->
    outT[65, n] += [vT | 1]^T expT   (row 64 = softmax denominator)
  softmax normalization via exp(-ln(sumexp) + ln(2*gamma)) on ACT and a
  K=1 ones-matmul that broadcasts it across partitions; residual fused:
    out = x + 2*y + 2*gamma*(attn_out + v_b)

Conv tricks: every pass streams ONE contiguous span of the zero-padded
[66x66] image (row-tap start offsets keep passes column-aligned in
PSUM; pad-column garbage is skipped at SiLU evacuation), keeping the PE
at its 1 column/cycle fp16 rate. Column taps ride the CONTRACTION axis
instead of costing extra passes: conv1 streams host-built stacks
[x, x<<1] and [x<<2, x<<68] so its 576-element contraction runs in the
minimal 5 passes/tile (4x K=128 + 1x K=64, down from 9); conv2 stacks
[y1, y1<<1, y1<<2] on 96 partitions (two on-chip shifted DMA copies) so
all 3 column taps fuse into K=96 (3 passes instead of 9).
"""

import sys

sys.path.insert(0, "/opt/trn_rl_repo")

from contextlib import ExitStack

import numpy as np
import ml_dtypes

import concourse.bass as bass
import concourse.tile as tile
from concourse import bacc, mybir
from concourse.bass_utils import run_bass_kernel_spmd

B, C1, C2, Cm, C8 = 4, 64, 64, 32, 8
H = W = 64
N = H * W            # 4096 pixels
NH = N // 2          # 2048 pixels per core (32 rows)
HP = H + 2           # padded height
WP = W + 2
NP = HP * WP         # 4356
NCORES = 8
EPS = 1e-5
FP32 = mybir.dt.float32
AF = mybir.ActivationFunctionType
ALU = mybir.AluOpType

MCHUNKS = N // 128   # 32 attention m-chunks
NSPAN = 1024         # n columns processed per accumulator half
BF16 = mybir.dt.float16  # 16-bit matmul operand dtype (fp16: 8x less rounding than bf16)
FP8 = mybir.dt.float8e4   # e4m3 for the attention-weights matmul (DoubleRow)
VP = 80                   # padded per-chunk lhsT columns (65 -> 80, step%16==0)
RPT = 7              # conv: image rows per matmul (contiguous-stream tiling)

_build_cache = {}


def _build_program():
    if "nc" in _build_cache:
        return _build_cache["nc"]
    nc = bacc.Bacc("TRN2", target_bir_lowering=False, debug=False, num_devices=NCORES)

    xp_d = nc.dram_tensor("xs", [128, NP], BF16, kind="ExternalInput")
    xh_d = nc.dram_tensor("xh", [C1, NH], FP32, kind="ExternalInput")
    w1_d = nc.dram_tensor("w1a", [128, 3 * Cm], BF16, kind="ExternalInput")
    w1b_d = nc.dram_tensor("w1b", [C1, Cm], BF16, kind="ExternalInput")
    w1c_d = nc.dram_tensor("w1c", [128, Cm], BF16, kind="ExternalInput")
    xs2_d = nc.dram_tensor("xs2", [128, NP], BF16, kind="ExternalInput")
    b1_d = nc.dram_tensor("b1", [Cm, 1], FP32, kind="ExternalInput")
    w2_d = nc.dram_tensor("w2s", [96, 3 * C2], BF16, kind="ExternalInput")
    b2_d = nc.dram_tensor("b2", [C2, 1], FP32, kind="ExternalInput")
    qw_d = nc.dram_tensor("qwT", [C2, C8], BF16, kind="ExternalInput")
    qb_d = nc.dram_tensor("qb", [C8, 1], FP32, kind="ExternalInput")
    kw_d = nc.dram_tensor("kwT", [C2, C8], BF16, kind="ExternalInput")
    kb_d = nc.dram_tensor("kb", [C8, 1], FP32, kind="ExternalInput")
    vw_d = nc.dram_tensor("vwT", [C2, C2], BF16, kind="ExternalInput")
    g2_d = nc.dram_tensor("g2", [C2, 1], FP32, kind="ExternalInput")
    fb_d = nc.dram_tensor("fb", [C2, 1], FP32, kind="ExternalInput")
    lg_d = nc.dram_tensor("ln2g", [1, 1], FP32, kind="ExternalInput")
    out_d = nc.dram_tensor("out", [C2, NH], FP32, kind="ExternalOutput")

    with tile.TileContext(nc) as tc:
        with ExitStack() as ctx:
            per = ctx.enter_context(tc.tile_pool(name="persist", bufs=1))

            xs_sb = per.tile([128, NP], BF16)
            xh_sb = per.tile([C1, NH], FP32)
            w1_sb = per.tile([128, 3 * Cm], BF16)
            w1b_sb = per.tile([C1, Cm], BF16)
            w1c_sb = per.tile([128, Cm], BF16)
            xs2_sb = per.tile([128, NP], BF16)
            b1_sb = per.tile([Cm, 1], FP32)
            w2_sb = per.tile([96, 3 * C2], BF16)
            b2_sb = per.tile([C2, 1], FP32)
            qw_sb = per.tile([C2, C8], BF16)
            qb_sb = per.tile([C8, 1], FP32)
            kw_sb = per.tile([C2, C8], BF16)
            kb_sb = per.tile([C8, 1], FP32)
            vw_sb = per.tile([C2, C2], BF16)
            g2_sb = per.tile([C2, 1], FP32)
            fb_sb = per.tile([C2, 1], FP32)
            lg_sb = per.tile([1, 1], FP32)
            ones_sb = per.tile([1, C2], BF16)

            ys_sb = per.tile([96, NP], BF16)       # conv1 out + 2 column-shifted copies
            y_sb = per.tile([C2, N], BF16)         # conv2 output == yf
            k_sb = per.tile([C8, N], BF16)
            q_sb = per.tile([C8, NH], BF16)
            vext_sb = per.tile([128, (MCHUNKS // 2) * 2 * VP], FP8)  # [128, 16, 2, 80]
            r_sb = per.tile([C2, NH], FP32)        # x_half + 2*y_half
            fin_sb = per.tile([C2, NH], FP32)

            for sb, d in [
                (xs_sb, xp_d), (xs2_sb, xs2_d), (xh_sb, xh_d), (w1_sb, w1_d), (w1b_sb, w1b_d), (w1c_sb, w1c_d), (b1_sb, b1_d),
                (w2_sb, w2_d), (b2_sb, b2_d), (qw_sb, qw_d), (qb_sb, qb_d),
                (kw_sb, kw_d), (kb_sb, kb_d), (vw_sb, vw_d), (g2_sb, g2_d),
                (fb_sb, fb_d), (lg_sb, lg_d),
            ]:
                nc.sync.dma_start(sb[:], d.ap())

            nc.gpsimd.memset(ones_sb[:], 1.0)
            nc.gpsimd.memset(ys_sb[:], 0.0)
            vext_v = vext_sb[:].rearrange("p (c s k) -> p c s k", s=2, k=VP)
            nc.gpsimd.memset(vext_sb[:], 0.0)
            nc.gpsimd.memset(vext_v[:, :, :, C2:C2 + 1], 1.0)

            ys_v = ys_sb[:].rearrange("p (a b) -> p a b", b=WP)
            y_v = y_sb[:]
            y_rows = y_sb[:].rearrange("p (a b) -> p a b", b=W)

            # conv tiling: groups of RPT image rows; each tap streams one
            # CONTIGUOUS span of the padded image (garbage at the 2 pad
            # columns per row accumulates in psum and is skipped on
            # evacuation).
            conv_tiles = [(RPT * t, RPT) for t in range(H // RPT)]
            if H % RPT:
                conv_tiles.append((H - H % RPT, H % RPT))

            # conv1: 5 streamed passes/tile (the contraction-lower-bound):
            # 3x K=128 on xs=[x, x<<1] (taps (u,0)+(u,1)), 1x K=128 on
            # xs2=[x<<2, x<<68] (taps (0,2)+(1,2)), 1x K=64 on xs2 at a
            # +2*WP offset (tap (2,2)).
            def conv1_tile(psA, r0, nr):
                length = WP * (nr - 1) + W
                ps = psA.tile([Cm, WP * nr], FP32, tag="mm")
                for u in range(3):
                    s = (r0 + u) * WP
                    nc.tensor.matmul(
                        ps[:, 0:length], w1_sb[:, Cm * u:Cm * (u + 1)],
                        xs_sb[:, s:s + length], start=(u == 0), stop=False,
                    )
                s = r0 * WP
                nc.tensor.matmul(
                    ps[:, 0:length], w1c_sb[:], xs2_sb[:, s:s + length],
                    start=False, stop=False,
                )
                nc.tensor.matmul(
                    ps[:, 0:length], w1b_sb[:],
                    xs2_sb[0:C1, s + 2 * WP:s + 2 * WP + length],
                    start=False, stop=True,
                )
                ps_v = ps[:].rearrange("p (r w) -> p r w", w=WP)
                nc.scalar.activation(
                    ys_v[0:Cm, 1 + r0:1 + r0 + nr, 1:1 + W], ps_v[:, 0:nr, 0:W],
                    AF.Silu, bias=b1_sb[:, 0:1],
                )

            # conv2: all 3 column taps on the partition axis (K=96, shifted
            # copies of y1 at rows 32-63 / 64-95): 3 passes instead of 9.
            def conv2_tile(psA, r0, nr):
                length = WP * (nr - 1) + W
                ps = psA.tile([C2, WP * nr], FP32, tag="mm")
                for u in range(3):
                    s = (r0 + u) * WP
                    nc.tensor.matmul(
                        ps[:, 0:length], w2_sb[:, C2 * u:C2 * (u + 1)],
                        ys_sb[:, s:s + length], start=(u == 0), stop=(u == 2),
                    )
                ps_v = ps[:].rearrange("p (r w) -> p r w", w=WP)
                nc.scalar.activation(
                    y_rows[:, r0:r0 + nr, :], ps_v[:, 0:nr, 0:W],
                    AF.Silu, bias=b2_sb[:, 0:1],
                )

            with tc.tile_pool(name="psA", bufs=4, space="PSUM") as psA:
                for r0, nr in conv_tiles:
                    conv1_tile(psA, r0, nr)
                # build the column-shifted y1 copies (cross-partition DMA)
                nc.sync.dma_start(ys_sb[Cm:2 * Cm, 0:NP - 1], ys_sb[0:Cm, 1:NP])
                nc.sync.dma_start(ys_sb[2 * Cm:3 * Cm, 0:NP - 2], ys_sb[0:Cm, 2:NP])
                for r0, nr in conv_tiles:
                    conv2_tile(psA, r0, nr)

                # ---- k = kwT.T @ y + kb  (full N) ----
                for t in range(N // 512):
                    ps = psA.tile([C8, 512], FP32, tag="mm")
                    nc.tensor.matmul(ps[:], kw_sb[:], y_v[:, 512 * t:512 * (t + 1)],
                                     start=True, stop=True)
                    nc.vector.tensor_scalar_add(k_sb[:, 512 * t:512 * (t + 1)],
                                                ps[:], kb_sb[:, 0:1])

                # ---- q = qwT.T @ y[:, :NH] + qb ----
                for t in range(NH // 512):
                    ps = psA.tile([C8, 512], FP32, tag="mm")
                    nc.tensor.matmul(ps[:], qw_sb[:], y_v[:, 512 * t:512 * (t + 1)],
                                     start=True, stop=True)
                    nc.vector.tensor_scalar_add(q_sb[:, 512 * t:512 * (t + 1)],
                                                ps[:], qb_sb[:, 0:1])

                # ---- vT[n, c] = y[:, n].T @ vwT ; packed 8 chunks per psum ----
                for g in range(MCHUNKS // 8):
                    ps = psA.tile([128, 512], FP32, tag="mm")
                    for i in range(8):
                        j = 8 * g + i
                        nc.tensor.matmul(
                            ps[:, C2 * i:C2 * (i + 1)],
                            y_v[:, 128 * j:128 * (j + 1)],
                            vw_sb[:],
                            start=True, stop=True,
                        )
                    nc.vector.tensor_copy(vext_v[:, 4 * g:4 * (g + 1), :, 0:C2], ps[:])

                # ---- residual: r = 2*y_half + x_half ----
                nc.vector.scalar_tensor_tensor(
                    r_sb[:], y_v[:, 0:NH], 2.0, xh_sb[:], ALU.mult, ALU.add,
                )

            # ---- attention, two sequential 1024-col halves ----
            with (
                tc.tile_pool(name="psE", bufs=2, space="PSUM") as psE,
                tc.tile_pool(name="psO", bufs=2, space="PSUM") as psO,
                tc.tile_pool(name="expp", bufs=3) as expp,
            ):
                for nh in range(2):
                    cs = NSPAN * nh
                    po = psO.tile([VP, NSPAN], FP32, tag="po")
                    pending = []
                    for p in range(MCHUNKS // 2):
                        exv = expp.tile([128, 2 * NSPAN], FP8, tag="ex")
                        exv = exv[:].rearrange("q (s n) -> q s n", s=2)
                        for s in range(2):
                            i = 2 * p + s
                            pe = psE.tile([128, NSPAN], FP32, tag="pe")
                            for j in range(2):
                                nc.tensor.matmul(
                                    pe[:, 512 * j:512 * (j + 1)],
                                    k_sb[:, 128 * i:128 * (i + 1)],
                                    q_sb[:, cs + 512 * j:cs + 512 * (j + 1)],
                                    start=True, stop=True,
                                )
                            nc.scalar.activation(exv[:, s, :], pe[:], AF.Exp)
                        pending.append((exv, p))
                        if len(pending) > 2:
                            _mm2(nc, po, vext_v, *pending.pop(0))
                    for item in pending:
                        _mm2(nc, po, vext_v, *item)

                    # ---- epilogue for this half ----
                    # epilogue pipelined in 512-col blocks across ACT/PE/DVE
                    o_sb = per.tile([C2, NSPAN], FP32, tag="o_sb")
                    lnr = per.tile([1, NSPAN], FP32, tag="lnr")
                    rec = per.tile([1, NSPAN], BF16, tag="rec")
                    t1 = per.tile([C2, NSPAN], FP32, tag="t1")
                    for j in range(2):
                        sl = slice(512 * j, 512 * (j + 1))
                        gl = slice(cs + 512 * j, cs + 512 * (j + 1))
                        nc.scalar.activation(lnr[:, sl], po[C2:C2 + 1, sl], AF.Ln)
                        nc.vector.tensor_copy(o_sb[:, sl], po[0:C2, sl])
                        # rec = exp(-ln(sumexp) + ln(2*gamma)) = 2*gamma / sumexp
                        nc.scalar.activation(rec[:, sl], lnr[:, sl], AF.Exp,
                                             scale=-1.0, bias=lg_sb[0:1, 0:1])
                        nc.tensor.matmul(po[0:C2, sl], ones_sb[:], rec[:, sl],
                                         start=True, stop=True)
                        nc.vector.tensor_mul(t1[:, sl], o_sb[:, sl], po[0:C2, sl])
                        nc.vector.scalar_tensor_tensor(
                            fin_sb[:, gl], t1[:, sl], fb_sb[:, 0:1], r_sb[:, gl],
                            ALU.add, ALU.add,
                        )
                        nc.sync.dma_start(out_d.ap()[:, gl], fin_sb[:, gl])

    nc.compile()
    _build_cache["nc"] = nc
    return nc


def _mm2(nc, po, vext_v, exv, p):
    # DoubleRow fp8: contract 256 m-rows (chunk pair 2p, 2p+1) per pass.
    # po[m, n] += sum_s vext_{2p+s}[:, m]^T expT_{2p+s}[:, n]; row C2 = sum(exp)
    for j in range(2):
        nc.tensor.matmul(
            po[:, 512 * j:512 * (j + 1)],
            vext_v[:, p, :, :],
            exv[:, :, 512 * j:512 * (j + 1)],
            start=(p == 0), stop=(p == MCHUNKS // 2 - 1),
            perf_mode=mybir.MatmulPerfMode.DoubleRow,
        )


def _host_prep(inputs):
    f32 = np.float32
    x = np.asarray(inputs["x"], f32)
    s1 = np.asarray(inputs["bn1_g"], f32) / np.sqrt(np.asarray(inputs["bn1_v"], f32) + EPS)
    bb1 = np.asarray(inputs["bn1_b"], f32) - np.asarray(inputs["bn1_m"], f32) * s1
    w1 = np.asarray(inputs["cv1_w"], f32) * s1[:, None, None, None]
    s2 = np.asarray(inputs["bn2_g"], f32) / np.sqrt(np.asarray(inputs["bn2_v"], f32) + EPS)
    bb2 = np.asarray(inputs["bn2_b"], f32) - np.asarray(inputs["bn2_m"], f32) * s2
    w2 = np.asarray(inputs["cv2_w"], f32) * s2[:, None, None, None]
    gamma = f32(np.asarray(inputs["pam_gamma"], f32))

    def wt(w, cin, cout):
        # [cout, cin, 3, 3] -> [cin, 9*cout], tap-major column blocks
        return np.ascontiguousarray(
            w.transpose(2, 3, 1, 0).reshape(9, cin, cout).transpose(1, 0, 2).reshape(cin, 9 * cout)
        )

    common = {
        "b1": np.ascontiguousarray(bb1[:, None]),
        "b2": np.ascontiguousarray(bb2[:, None]),
        "qwT": np.ascontiguousarray(np.asarray(inputs["q_w"], f32).T).astype(np.float16),
        "qb": np.ascontiguousarray(np.asarray(inputs["q_b"], f32)[:, None]),
        "kwT": np.ascontiguousarray(np.asarray(inputs["k_w"], f32).T).astype(np.float16),
        "kb": np.ascontiguousarray(np.asarray(inputs["k_b"], f32)[:, None]),
        "vwT": np.ascontiguousarray(np.asarray(inputs["v_w"], f32).T).astype(np.float16),
        "g2": np.full((C2, 1), 2.0 * gamma, f32),
        "ln2g": np.full((1, 1), np.log(2.0 * gamma), f32),
        "fb": np.ascontiguousarray((2.0 * gamma * np.asarray(inputs["v_b"], f32))[:, None]),
    }
    bf = np.float16

    def packs(w1f, w2f):
        a = np.zeros((128, 3 * Cm), np.float32)
        s2 = np.zeros((96, 3 * C2), np.float32)
        c = np.zeros((128, Cm), np.float32)
        for u in range(3):
            a[0:C1, Cm * u:Cm * (u + 1)] = w1f[:, :, u, 0].T
            a[C1:128, Cm * u:Cm * (u + 1)] = w1f[:, :, u, 1].T
            for j in range(3):
                s2[Cm * j:Cm * (j + 1), C2 * u:C2 * (u + 1)] = w2f[:, :, u, j].T
        c[0:C1, :] = w1f[:, :, 0, 2].T
        c[C1:128, :] = w1f[:, :, 1, 2].T
        b = np.ascontiguousarray(w1f[:, :, 2, 2].T)
        return a.astype(bf), b.astype(bf), c.astype(bf), s2.astype(bf)

    wp = {0: packs(w1, w2), 1: packs(w1[:, :, ::-1, :], w2[:, :, ::-1, :])}

    in_maps = []
    for core in range(NCORES):
        b, fl = core // 2, core % 2
        xb = x[b] if fl == 0 else x[b][:, ::-1, :]
        xpad = np.zeros((C1, HP, WP), f32)
        xpad[:, 1:H + 1, 1:W + 1] = xb
        m = dict(common)
        xpf = xpad.reshape(C1, NP).astype(np.float16)
        sh1 = np.zeros_like(xpf); sh1[:, :-1] = xpf[:, 1:]
        sh2 = np.zeros_like(xpf); sh2[:, :-2] = xpf[:, 2:]
        sh68 = np.zeros_like(xpf); sh68[:, :-68] = xpf[:, 68:]
        m["xs"] = np.concatenate([xpf, sh1], axis=0)
        m["xs2"] = np.concatenate([sh2, sh68], axis=0)
        m["xh"] = np.ascontiguousarray(xb[:, 0:H // 2, :].reshape(C1, NH))
        m["w1a"], m["w1b"], m["w1c"], m["w2s"] = wp[fl]
        in_maps.append(m)
    return in_maps


def _assemble(results):
    out = np.empty((B, C2, H, W), np.float32)
    for core in range(NCORES):
        b, fl = core // 2, core % 2
        o = results[core]["out"].reshape(C2, H // 2, W)
        if fl == 0:
            out[b, :, 0:H // 2, :] = o
        else:
            out[b, :, H // 2:H, :] = o[:, ::-1, :]
    return out


def _run(inputs, trace=False):
    nc = _build_program()
    in_maps = _host_prep(inputs)
    res = run_bass_kernel_spmd(nc, in_maps, core_ids=list(range(NCORES)), trace=trace)
    return _assemble(res.results), res


def kernel(**inputs):
    out, _ = _run(inputs)
    return out
